# revision 8
# baseline (speedup 1.0000x reference)
"""PaiNN message-passing GNN on 8 Trainium2 NeuronCores.

Strategy (graph/data parallel, atom-sharded):
  - Core c owns atoms [c*1024, (c+1)*1024). Pairs are bucketed by owner of
    idx_i, sorted, grouped into 8 windows of 128 atoms, padded per window.
  - Atom state: q kept feature-major (qT [128, 1024]), mu kept atom-major
    (mu_am [128p, 8w, 384]) + transposed per block for the mixing matmuls.
  - Per block: interaction MLP on the shard -> AllGather (x||mu) rows into a
    shared HBM table -> indirect-DMA gather of x[idx_j]||mu[idx_j] ->
    messages on DVE -> scatter-add via one-hot matmul into PSUM (local atoms
    only, no cross-core scatter) -> mixing MLP on the shard.
  - Padded pairs carry cutoff=0 so their descriptor row is 0 and they
    contribute exactly nothing.
Outputs are per-core shards; the host concatenates/transposes.
"""
import sys

sys.path.insert(0, "/opt/trn_rl_repo")
import numpy as np

import concourse.bacc as bacc
import concourse.bass as bass
import concourse.mybir as mybir
import concourse.tile as tile
from concourse.bass_utils import run_bass_kernel_spmd
from concourse.masks import make_identity

import os
NCORES = 8
NATOMS = 8192
F = 128
RB = 20
NBLK = 5
DBG_NBLK = int(os.environ.get("KNBLK", NBLK))   # blocks to actually run
DBG_SKIPMIX = bool(int(os.environ.get("KSKIPMIX", "0")))
DBG_SKIPPAIR = bool(int(os.environ.get("KSKIPPAIR", "0")))
P = 128
SHARD = NATOMS // NCORES          # 1024
NW = SHARD // P                   # 8 windows of 128 atoms
EPS = 1e-8
FP = mybir.dt.float32

_compiled = {}


def _build(NT, TPW):
    """NT = pair tiles per core (NW*TPW), TPW = tiles per window."""
    nc = bacc.Bacc(None, target_bir_lowering=False)

    # ---- I/O ----
    qT_in = nc.dram_tensor("qT_in", [P, SHARD], FP, kind="ExternalInput")
    pconst = nc.dram_tensor("pconst", [P, NT, 5], FP, kind="ExternalInput")
    dist_in = nc.dram_tensor("dist_in", [P, NT], FP, kind="ExternalInput")
    idxj_in = nc.dram_tensor("idxj_in", [P, NT], mybir.dt.int32, kind="ExternalInput")
    rbfsT_in = nc.dram_tensor("rbfsT_in", [RB, NT * P], FP, kind="ExternalInput")
    filtW_in = nc.dram_tensor("filtW_in", [RB, NBLK * 3 * F], FP, kind="ExternalInput")
    intW1_in = nc.dram_tensor("intW1_in", [NBLK, F, F], FP, kind="ExternalInput")
    intW2_in = nc.dram_tensor("intW2_in", [NBLK, F, 3 * F], FP, kind="ExternalInput")
    mixWv_in = nc.dram_tensor("mixWv_in", [NBLK, F, 2 * F], FP, kind="ExternalInput")
    mixW1_in = nc.dram_tensor("mixW1_in", [NBLK, 2 * F, F], FP, kind="ExternalInput")
    mixW2_in = nc.dram_tensor("mixW2_in", [NBLK, F, 3 * F], FP, kind="ExternalInput")

    qT_out = nc.dram_tensor("qT_out", [P, SHARD], FP, kind="ExternalOutput")
    mu_out = nc.dram_tensor("mu_out", [SHARD, 3 * F], FP, kind="ExternalOutput")

    ag_in = nc.dram_tensor("ag_in", [SHARD, 768], FP, kind="Internal")
    table = nc.dram_tensor("table", [NATOMS, 768], FP, kind="Internal",
                           addr_space="Shared")

    with tile.TileContext(nc) as tc:
        with (
            tc.tile_pool(name="state", bufs=1) as st,
            tc.tile_pool(name="work", bufs=2) as wk,
            tc.tile_pool(name="gather", bufs=4) as gp,
            tc.tile_pool(name="small", bufs=2) as sm,
            tc.tile_pool(name="ps_big", bufs=1, space="PSUM") as ps_big,
            tc.tile_pool(name="ps_acc", bufs=1, space="PSUM") as ps_acc,
            tc.tile_pool(name="ps_desc", bufs=2, space="PSUM") as ps_desc,
            tc.tile_pool(name="ps_tr", bufs=1, space="PSUM") as ps_tr,
        ):
            # ---- persistent state / constants ----
            qT = st.tile([P, SHARD], FP)            # feature-major q
            mu_am = st.tile([P, NW, 3 * F], FP)     # atom-major mu rows
            muT = st.tile([P, 3, SHARD], FP)        # feature-major mu (rebuilt)
            x_am = st.tile([P, NW, 3 * F], FP)      # atom-major x rows
            Sall = st.tile([P, NT, P], FP)          # one-hot scatter tiles
            pc = st.tile([P, NT, 5], FP)            # cut,dirx,diry,dirz,idxiloc
            idxj = st.tile([P, NT], mybir.dt.int32)
            filtW = st.tile([RB, NBLK * 3 * F], FP)
            intW1 = st.tile([P, NBLK, F], FP)
            intW2 = st.tile([P, NBLK, 3 * F], FP)
            mixWv = st.tile([P, NBLK, 2 * F], FP)
            mixW1 = st.tile([P, NBLK, 2, F], FP)    # K-chunks on dim 2
            mixW2 = st.tile([P, NBLK, 3 * F], FP)
            ident = st.tile([P, P], FP)
            muV = st.tile([P, 3, SHARD], FP)
            y1s = st.tile([P, SHARD], FP)
            muVn = st.tile([P, SHARD], FP)
            dmu_i = st.tile([P, SHARD], FP)
            dqmu_i = st.tile([P, SHARD], FP)
            scal = st.tile([P, SHARD], FP)
            tmp1 = st.tile([P, SHARD], FP)
            nsq = st.tile([P, SHARD], FP)
            eps_t = st.tile([P, 1], FP)

            nc.vector.memset(eps_t[:], EPS)
            make_identity(nc, ident[:])
            nc.sync.dma_start(out=qT[:], in_=qT_in[:])
            nc.sync.dma_start(out=pc[:], in_=pconst[:])
            nc.sync.dma_start(out=idxj[:], in_=idxj_in[:])
            nc.sync.dma_start(out=filtW[:], in_=filtW_in[:])
            nc.sync.dma_start(out=intW1[:], in_=intW1_in.rearrange("b k f -> k b f"))
            nc.sync.dma_start(out=intW2[:], in_=intW2_in.rearrange("b k f -> k b f"))
            nc.sync.dma_start(out=mixWv[:], in_=mixWv_in.rearrange("b k f -> k b f"))
            nc.sync.dma_start(out=mixW1[:],
                              in_=mixW1_in.rearrange("b (c k) f -> k b c f", k=P))
            nc.sync.dma_start(out=mixW2[:], in_=mixW2_in.rearrange("b k f -> k b f"))
            nc.vector.memset(mu_am[:], 0.0)

            # dir = vectors / distances  (overwrite pc[:, :, 1:4])
            dist = sm.tile([P, NT], FP, tag="dist")
            nc.sync.dma_start(out=dist[:], in_=dist_in[:])
            recip = sm.tile([P, NT], FP, tag="dist")
            nc.vector.reciprocal(recip[:], dist[:])
            for d in range(3):
                nc.vector.tensor_tensor(out=pc[:, :, 1 + d], in0=pc[:, :, 1 + d],
                                        in1=recip[:], op=mybir.AluOpType.mult)

            # one-hot S tiles from local atom index: S[r, a] = (idxiloc[r]==a)
            iota_i = sm.tile([P, P], mybir.dt.int32, tag="iota")
            iota_f = sm.tile([P, P], FP, tag="iotaf")
            nc.gpsimd.iota(iota_i[:], pattern=[[1, P]], base=0, channel_multiplier=0)
            nc.vector.tensor_copy(out=iota_f[:], in_=iota_i[:])
            for t in range(NT):
                nc.vector.tensor_tensor(
                    out=Sall[:, t, :],
                    in0=pc[:, t, 4:5].to_broadcast([P, P]),
                    in1=iota_f[:], op=mybir.AluOpType.is_equal)

            for b in range(DBG_NBLK):
                # ---------- interaction MLP: x = silu(q@W1)@W2 ----------
                y1ps = ps_big.tile([P, SHARD], FP, tag="big")
                for h in range(2):
                    nc.tensor.matmul(out=y1ps[:, h * 512:(h + 1) * 512],
                                     lhsT=intW1[:, b, :],
                                     rhs=qT[:, h * 512:(h + 1) * 512],
                                     start=True, stop=True)
                nc.scalar.activation(out=y1s[:], in_=y1ps[:],
                                     func=mybir.ActivationFunctionType.Silu)
                for t in range(NW):
                    xps = ps_desc.tile([P, 3 * F], FP, tag="desc")
                    nc.tensor.matmul(out=xps[:], lhsT=y1s[:, t * P:(t + 1) * P],
                                     rhs=intW2[:, b, :], start=True, stop=True)
                    nc.vector.tensor_copy(out=x_am[:, t, :], in_=xps[:])

                # ---------- publish x||mu rows, AllGather table ----------
                nc.sync.dma_start(
                    out=ag_in.rearrange("(w p) f -> p w f", p=P)[:, :, 0:384],
                    in_=x_am[:])
                nc.sync.dma_start(
                    out=ag_in.rearrange("(w p) f -> p w f", p=P)[:, :, 384:768],
                    in_=mu_am[:])
                nc.gpsimd.collective_compute(
                    "AllGather", mybir.AluOpType.bypass,
                    replica_groups=[list(range(NCORES))],
                    ins=[ag_in[:]], outs=[table[:]])

                # ---------- pair phase ----------
                for w in range(0 if not DBG_SKIPPAIR else NW, NW):
                    acc = ps_acc.tile([P, 384], FP, tag="acc")
                    accq = ps_acc.tile([P, P], FP, tag="accq")
                    for k in range(TPW):
                        t = w * TPW + k
                        # descriptors for this tile (cutoff folded via ACT)
                        rb_sb = wk.tile([RB, P], FP, tag="rbf")
                        nc.sync.dma_start(out=rb_sb[:],
                                          in_=rbfsT_in[:, t * P:(t + 1) * P])
                        dps = ps_desc.tile([P, 3 * F], FP, tag="desc")
                        nc.tensor.matmul(out=dps[:], lhsT=rb_sb[:],
                                         rhs=filtW[:, b * 384:(b + 1) * 384],
                                         start=True, stop=True)
                        desc = wk.tile([P, 3 * F], FP, tag="descs")
                        nc.scalar.activation(out=desc[:], in_=dps[:],
                                             func=mybir.ActivationFunctionType.Copy,
                                             scale=pc[:, t, 0:1])
                        # gather x[idx_j] || mu[idx_j]
                        g = gp.tile([P, 768], FP, tag="g")
                        nc.gpsimd.indirect_dma_start(
                            out=g[:], out_offset=None, in_=table[:],
                            in_offset=bass.IndirectOffsetOnAxis(
                                ap=idxj[:, t:t + 1], axis=0))
                        # messages
                        xjd = wk.tile([P, 3 * F], FP, tag="xjd")
                        nc.vector.tensor_tensor(out=xjd[:], in0=g[:, 0:384],
                                                in1=desc[:], op=mybir.AluOpType.mult)
                        dmu = wk.tile([P, 3 * F], FP, tag="dmu")
                        for d in range(3):
                            td = wk.tile([P, F], FP, tag="td")
                            nc.gpsimd.tensor_tensor(
                                out=td[:], in0=xjd[:, 256:384],
                                in1=g[:, 384 + d * F:384 + (d + 1) * F],
                                op=mybir.AluOpType.mult)
                            nc.vector.scalar_tensor_tensor(
                                out=dmu[:, d * F:(d + 1) * F],
                                in0=xjd[:, 128:256], scalar=pc[:, t, 1 + d:2 + d],
                                in1=td[:],
                                op0=mybir.AluOpType.mult, op1=mybir.AluOpType.add)
                        # scatter-add via one-hot matmul
                        nc.tensor.matmul(out=accq[:], lhsT=Sall[:, t, :],
                                         rhs=xjd[:, 0:128],
                                         start=(k == 0), stop=(k == TPW - 1))
                        nc.tensor.matmul(out=acc[:], lhsT=Sall[:, t, :],
                                         rhs=dmu[:],
                                         start=(k == 0), stop=(k == TPW - 1))
                    # apply window updates
                    nc.vector.tensor_add(out=mu_am[:, w, :], in0=mu_am[:, w, :],
                                         in1=acc[:])
                    dq_sb = sm.tile([P, P], FP, tag="dqsb")
                    nc.scalar.activation(out=dq_sb[:], in_=accq[:],
                                         func=mybir.ActivationFunctionType.Copy)
                    trp = ps_tr.tile([P, P], FP, tag="tr")
                    nc.tensor.transpose(out=trp[:], in_=dq_sb[:], identity=ident[:])
                    nc.vector.tensor_add(out=qT[:, w * P:(w + 1) * P],
                                         in0=qT[:, w * P:(w + 1) * P], in1=trp[:])

                # ---------- mixing ----------
                if DBG_SKIPMIX:
                    continue
                # muT = transpose(mu_am)
                for d in range(3):
                    for t in range(NW):
                        trp = ps_tr.tile([P, P], FP, tag="tr")
                        nc.tensor.transpose(out=trp[:],
                                            in_=mu_am[:, t, d * F:(d + 1) * F],
                                            identity=ident[:])
                        nc.vector.tensor_copy(out=muT[:, d, t * P:(t + 1) * P],
                                              in_=trp[:])
                # mu_V (feature-major)
                for d in range(3):
                    vps = ps_big.tile([P, SHARD], FP, tag="big")
                    for h in range(2):
                        nc.tensor.matmul(out=vps[:, h * 512:(h + 1) * 512],
                                         lhsT=mixWv[:, b, 0:128],
                                         rhs=muT[:, d, h * 512:(h + 1) * 512],
                                         start=True, stop=True)
                    nc.vector.tensor_copy(out=muV[:, d, :], in_=vps[:])
                # norm
                nc.vector.tensor_tensor(out=nsq[:], in0=muV[:, 0, :],
                                        in1=muV[:, 0, :], op=mybir.AluOpType.mult)
                for d in (1, 2):
                    nc.vector.tensor_tensor(out=tmp1[:], in0=muV[:, d, :],
                                            in1=muV[:, d, :], op=mybir.AluOpType.mult)
                    nc.vector.tensor_add(out=nsq[:], in0=nsq[:], in1=tmp1[:])
                nc.scalar.activation(out=muVn[:], in_=nsq[:],
                                     func=mybir.ActivationFunctionType.Sqrt,
                                     bias=eps_t[:])
                # mix MLP: y1 = silu([q, muVn] @ W1)
                y1ps = ps_big.tile([P, SHARD], FP, tag="big")
                for h in range(2):
                    sl = slice(h * 512, (h + 1) * 512)
                    nc.tensor.matmul(out=y1ps[:, sl], lhsT=mixW1[:, b, 0, :],
                                     rhs=qT[:, sl], start=True, stop=False)
                    nc.tensor.matmul(out=y1ps[:, sl], lhsT=mixW1[:, b, 1, :],
                                     rhs=muVn[:, sl], start=False, stop=True)
                nc.scalar.activation(out=y1s[:], in_=y1ps[:],
                                     func=mybir.ActivationFunctionType.Silu)
                # y planes: dq_i, dmu_i, dqmu_i
                for j, dst in ((0, tmp1), (1, dmu_i), (2, dqmu_i)):
                    yps = ps_big.tile([P, SHARD], FP, tag="big")
                    for h in range(2):
                        sl = slice(h * 512, (h + 1) * 512)
                        nc.tensor.matmul(out=yps[:, sl],
                                         lhsT=mixW2[:, b, j * F:(j + 1) * F],
                                         rhs=y1s[:, sl], start=True, stop=True)
                    nc.vector.tensor_copy(out=dst[:], in_=yps[:])
                # qT += dq_i  (tmp1 holds dq_i)
                nc.vector.tensor_add(out=qT[:], in0=qT[:], in1=tmp1[:])
                # muW per d: scal accum + mu update
                nc.vector.memset(scal[:], 0.0)
                for d in range(3):
                    wps = ps_big.tile([P, SHARD], FP, tag="big")
                    for h in range(2):
                        nc.tensor.matmul(out=wps[:, h * 512:(h + 1) * 512],
                                         lhsT=mixWv[:, b, 128:256],
                                         rhs=muT[:, d, h * 512:(h + 1) * 512],
                                         start=True, stop=True)
                    nc.vector.tensor_tensor(out=tmp1[:], in0=muV[:, d, :],
                                            in1=wps[:], op=mybir.AluOpType.mult)
                    nc.vector.tensor_add(out=scal[:], in0=scal[:], in1=tmp1[:])
                    # dmu_mix_d = dmu_i * muW_d  -> transpose into mu_am
                    nc.vector.tensor_tensor(out=tmp1[:], in0=dmu_i[:],
                                            in1=wps[:], op=mybir.AluOpType.mult)
                    for t in range(NW):
                        trp = ps_tr.tile([P, P], FP, tag="tr")
                        nc.tensor.transpose(out=trp[:],
                                            in_=tmp1[:, t * P:(t + 1) * P],
                                            identity=ident[:])
                        nc.vector.tensor_add(out=mu_am[:, t, d * F:(d + 1) * F],
                                             in0=mu_am[:, t, d * F:(d + 1) * F],
                                             in1=trp[:])
                # qT += dqmu_i * scal
                nc.vector.tensor_tensor(out=tmp1[:], in0=dqmu_i[:], in1=scal[:],
                                        op=mybir.AluOpType.mult)
                nc.vector.tensor_add(out=qT[:], in0=qT[:], in1=tmp1[:])

            nc.sync.dma_start(out=qT_out[:], in_=qT[:])
            nc.sync.dma_start(out=mu_out.rearrange("(w p) f -> p w f", p=P),
                              in_=mu_am[:])
    nc.compile()
    return nc


def _prep(inputs):
    """Host-side sharding: bucket pairs by owner of idx_i, pad per window."""
    idx_i = np.asarray(inputs["idx_i"]).astype(np.int64)
    idx_j = np.asarray(inputs["idx_j"]).astype(np.int64)
    rbfs = np.asarray(inputs["rbfs"], dtype=np.float32)
    cut = np.asarray(inputs["cutoffs"], dtype=np.float32)
    vec = np.asarray(inputs["vectors"], dtype=np.float32)
    dist = np.asarray(inputs["distances"], dtype=np.float32)

    owner = idx_i // SHARD
    win = (idx_i % SHARD) // P
    bucket = owner * NW + win
    order = np.argsort(bucket, kind="stable")
    counts = np.bincount(bucket, minlength=NCORES * NW)
    TPW = int(np.ceil(counts.max() / P))
    NT = NW * TPW
    NPAD = NT * P

    starts = np.zeros(NCORES * NW + 1, dtype=np.int64)
    np.cumsum(counts, out=starts[1:])

    per_core = []
    for c in range(NCORES):
        pcn = np.zeros((P, NT, 5), np.float32)
        dstw = np.ones((P, NT), np.float32)
        idxjw = np.zeros((P, NT), np.int32)
        rbT = np.zeros((RB, NPAD), np.float32)
        for w in range(NW):
            bidx = c * NW + w
            sel = order[starts[bidx]:starts[bidx + 1]]
            n = len(sel)
            s0 = w * TPW * P
            sl = np.arange(s0, s0 + n)
            r, t = sl % P, sl // P
            pcn[r, t, 0] = cut[sel]
            pcn[r, t, 1:4] = vec[sel]
            pcn[r, t, 4] = (idx_i[sel] - c * SHARD - w * P).astype(np.float32)
            dstw[r, t] = dist[sel]
            idxjw[r, t] = idx_j[sel]
            rbT[:, sl] = rbfs[sel].T
        per_core.append(dict(pconst=pcn, dist_in=dstw, idxj_in=idxjw, rbfsT_in=rbT))
    return per_core, NT, TPW


def kernel(**inputs):
    per_core, NT, TPW = _prep(inputs)
    key = (NT, TPW)
    if key not in _compiled:
        _compiled[key] = _build(NT, TPW)
    nc = _compiled[key]

    feats = np.asarray(inputs["features"], dtype=np.float32)
    filtW = np.asarray(inputs["filter_W"], dtype=np.float32)
    shared = dict(
        filtW_in=filtW,
        intW1_in=np.asarray(inputs["int_W1"], dtype=np.float32),
        intW2_in=np.asarray(inputs["int_W2"], dtype=np.float32),
        mixWv_in=np.asarray(inputs["mix_Wv"], dtype=np.float32),
        mixW1_in=np.asarray(inputs["mix_W1"], dtype=np.float32),
        mixW2_in=np.asarray(inputs["mix_W2"], dtype=np.float32),
    )
    in_maps = []
    for c in range(NCORES):
        m = dict(per_core[c])
        m.update(shared)
        m["qT_in"] = np.ascontiguousarray(feats[c * SHARD:(c + 1) * SHARD].T)
        in_maps.append(m)

    res = run_bass_kernel_spmd(nc, in_maps, list(range(NCORES)))
    q = np.concatenate([res.results[c]["qT_out"].T for c in range(NCORES)], axis=0)
    mu = np.concatenate(
        [res.results[c]["mu_out"].reshape(SHARD, 3, F) for c in range(NCORES)],
        axis=0)
    return q, mu
